# revision 36
# baseline (speedup 1.0000x reference)
"""MultiHeadAttention Trainium2 kernel (B=4, S=2048, D=1024, H=16, causal).

Sharding: 8 cores = batch(4) x head-group(2). Each core computes its batch's
attention for 8 heads (e-slice of 512) plus the partial out-projection for its
d-slice; host sums the two partials per batch and adds bo.

v3: attn@V runs "transposed" — out[q,65] tiles (N=65 moving columns) instead
of out[65,q] (N=512), halving its PE time since matmul cost is proportional
to the moving free size only. Causal skipping is exact at 128x128 tile
granularity for scores, exp, and attn@V (per-ktile qlo = 128*i on the diag
block); only the exact-diagonal 128-window needs a mask multiply. The [s,d]
attention output is normalized via per-partition reciprocal broadcast
(tensor_scalar_mul), then PE-transposed back to [d,s] for the out projection.
Projections are bf16 (same PE rate as float32r, half the DMA/SBUF).

Layouts (per core):
  QT/KT in SBUF as [e=512, s=2048] bf16 (4 partition tiles, head pair p in
       tile p: head 2p at partitions 0:64, 2p+1 at 64:128)
  V in SBUF as [s, 8 heads, 65] bf16 (64 v-cols + ones col for the denom)
  scores/exp in ST orientation ST[k, q]; ET = exp(ST/8) masked
  ps_o[q, j, qt, 65]: attn out + denom col, per q-chunk
  OTsd[s, h, hd] bf16 normalized -> PE transpose -> OTds[d, s] -> out proj
"""

import numpy as np

B, S, D, H = 4, 2048, 1024, 16
HD = D // H  # 64
NCORES = 8
HPG = 8          # heads per group (per core)
EP = HPG * HD    # 512, e-slice width per core
SCH = 512        # s-chunk width (q chunks, projection n chunks)
NSC = S // SCH   # 4
NST = S // 128   # 16 s-tiles
NDT = D // 128   # 8 d-tiles
NET = EP // 128  # 4 e-tiles per core (= head pairs)

_cache = {}


DEFAULT_OPTS = {
    "norm_perqt": False,   # normalize each q-tile as its stop lands
    "tp_dma": False,       # xbar DMA transpose instead of PE transpose
    "tp_in_block": False,  # transposes inline at block end vs phase end
    "pss_bufs": 2,
    "pso_bufs": 1,         # 2 needs tp_dma (frees the ps_t banks)
}


def _build_program(with_pad, with_bias=False, repeat=1, opts=None):
    from contextlib import ExitStack as _ExitStack

    import concourse.tile as tile
    from concourse import bacc, mybir
    from concourse.masks import make_identity

    o = dict(DEFAULT_OPTS)
    if opts:
        o.update(opts)

    f32 = mybir.dt.float32
    bdt = mybir.dt.bfloat16
    Exp = mybir.ActivationFunctionType.Exp

    def mm(out_ap, lhsT, rhs, **kw):
        nc.tensor.matmul(out_ap, lhsT, rhs, **kw)

    nc = bacc.Bacc("TRN2", target_bir_lowering=False)

    xT = nc.dram_tensor("xT", (D, S), bdt, kind="ExternalInput")
    wqT = nc.dram_tensor("wqT", (D, EP), bdt, kind="ExternalInput")
    wkT = nc.dram_tensor("wkT", (D, EP), bdt, kind="ExternalInput")
    wvT = nc.dram_tensor("wvT", (D, EP), bdt, kind="ExternalInput")
    woT = nc.dram_tensor("woT", (EP, D), bdt, kind="ExternalInput")
    if with_bias:
        bq = nc.dram_tensor("bq", (EP,), f32, kind="ExternalInput")
        bk = nc.dram_tensor("bk", (EP,), f32, kind="ExternalInput")
        bv = nc.dram_tensor("bv", (EP,), f32, kind="ExternalInput")
    cmask = nc.dram_tensor("cmask", (128, 2, 128), bdt, kind="ExternalInput")
    if with_pad:
        padm = nc.dram_tensor("padm", (NST, 128), f32, kind="ExternalInput")
    out = nc.dram_tensor("out", (S, D), bdt, kind="ExternalOutput")

    xT_t = xT.rearrange("(dt p) s -> p dt s", p=128)
    wqT_t = wqT.rearrange("(dt p) e -> p dt e", p=128)
    wkT_t = wkT.rearrange("(dt p) e -> p dt e", p=128)
    wvT_t = wvT.rearrange("(dt p) e -> p dt e", p=128)
    woT_t = woT.rearrange("(dt p) e -> p dt e", p=128)

    with tile.TileContext(nc) as tc:
        with tc.tile_pool(name="persist", bufs=1) as pp:
            # persistent SBUF tensors (live across phases)
            # Q and K interleaved (slot 0 = Q, 1 = K) so one copy per
            # proj unit moves both out of PSUM
            QKT = pp.tile([128, NET, 2, S], bdt)  # [e-tile, q/k, s]
            V = pp.tile([128, NST, HPG, HD + 1], bdt)  # ones col at index HD
            cm = pp.tile([128, 2, 128], bdt)   # exact-diag mask, [k, j, q]
            ident = pp.tile([128, 128], bdt)
            if with_bias:
                bq_sb = pp.tile([128, NET], f32)
                bk_sb = pp.tile([128, NET], f32)
                bv_row = pp.tile([1, EP], f32)
                bv_bc = pp.tile([128, HPG, HD], f32)
            if with_pad:
                pad_sb = pp.tile([128, NST], f32)
                nc.sync.dma_start(out=pad_sb, in_=padm.rearrange("t p -> p t"))

            nc.sync.dma_start(out=cm, in_=cmask[:, :, :])
            make_identity(nc, ident)
            if with_bias:
                nc.sync.dma_start(
                    out=bq_sb, in_=bq.rearrange("(t p) -> p t", p=128))
                nc.sync.dma_start(
                    out=bk_sb, in_=bk.rearrange("(t p) -> p t", p=128))
                nc.sync.dma_start(out=bv_row, in_=bv.rearrange("e -> 1 e"))
                nc.gpsimd.partition_broadcast(
                    bv_bc.rearrange("p h v -> p (h v)"), bv_row)
            for st in range(NST):
                nc.vector.memset(V[:, st, :, HD:HD + 1], 1.0)

            for _rep in range(repeat):
                # Fused schedule: projection chain-units of x-chunk sc are
                # interleaved with the attention blocks of q-chunk sc-1, so
                # the PE-bound projections fill the slack under the
                # ACT-bound exp stream instead of running serially before.
                with tc.tile_pool(name="ph", bufs=1) as lp, \
                     tc.tile_pool(name="phx", bufs=4) as xp, \
                     tc.tile_pool(name="et", bufs=5) as etp, \
                     tc.tile_pool(name="tmp", bufs=2) as tmpp, \
                     tc.tile_pool(name="fo", bufs=4) as fop, \
                     tc.tile_pool(name="ps_s", bufs=o["pss_bufs"],
                                  space="PSUM") as pss, \
                     tc.tile_pool(name="ps_o", bufs=o["pso_bufs"],
                                  space="PSUM") as pso, \
                     _ExitStack() as _psx:
                    pstp = (None if o["tp_dma"] else _psx.enter_context(
                        tc.tile_pool(name="ps_t", bufs=2, space="PSUM")))
                    OTds = lp.tile([128, NET, S], bdt)   # attn out, [d, s]
                    OTsd = lp.tile([128, NST, HPG, HD], bdt)  # attn out [s,d]
                    wo_sb = lp.tile([128, NET, D], bdt)
                    wq_sb = lp.tile([128, NDT, EP], bdt)
                    wk_sb = lp.tile([128, NDT, EP], bdt)
                    wv_sb = lp.tile([128, NDT, EP], bdt)
                    # DMA order = first-use order: the v units of phase 0
                    # need wv + x chunk 0 first, then qk needs wq/wk; x
                    # chunks 1-3 and wo stream in behind
                    xss = []
                    for sc in range(NSC):
                        xss.append(xp.tile([128, NDT, SCH], bdt, tag="xs",
                                           name=f"xs{sc}"))
                    # interleave wv/x0 per-dt so the first v-unit matmul
                    # (needs wv[0] + x0[0] only) starts ~0.5us in
                    for dt in range(NDT):
                        nc.sync.dma_start(
                            out=wv_sb[:, dt, :], in_=wvT_t[:, dt, :])
                        nc.sync.dma_start(
                            out=xss[0][:, dt, :],
                            in_=xT_t[:, dt, 0:SCH])
                    for dt in range(NDT):
                        nc.sync.dma_start(
                            out=wq_sb[:, dt, :], in_=wqT_t[:, dt, :])
                        nc.sync.dma_start(
                            out=wk_sb[:, dt, :], in_=wkT_t[:, dt, :])
                    for sc in range(1, NSC):
                        for dt in range(NDT):
                            nc.sync.dma_start(
                                out=xss[sc][:, dt, :],
                                in_=xT_t[:, dt, sc * SCH:(sc + 1) * SCH])
                        if sc == 1:
                            nc.sync.dma_start(out=wo_sb, in_=woT_t)

                    def proj_qk_unit(sc, xs, et):
                        # Q and K e-tile chains for chunk sc share one pss
                        # pair tile (Q slot 0, K slot 1); PSUM->SBUF copies
                        # convert to bf16 on the DVE
                        psqk = pss.tile([128, 2, SCH], f32, tag="pss")
                        for sl, w_sb in ((0, wq_sb), (1, wk_sb)):
                            for dt in range(NDT):
                                mm(
                                    psqk[:, sl, :],
                                    w_sb[:, dt, et * 128:(et + 1) * 128],
                                    xs[:, dt, :],
                                    start=(dt == 0), stop=(dt == NDT - 1))
                        if with_bias:
                            for sl in range(2):
                                nc.vector.tensor_scalar_add(
                                    QKT[:, et, sl, sc * SCH:(sc + 1) * SCH],
                                    psqk[:, sl, :],
                                    (bq_sb if sl == 0 else bk_sb)[:, et:et + 1])
                        else:
                            # DVE, not ACT: keep the ACT engine free for
                            # the exp stream it bottlenecks on
                            nc.vector.tensor_copy(
                                QKT[:, et, :, sc * SCH:(sc + 1) * SCH],
                                psqk[:, :, :])

                    def proj_v_unit(sc, xs, st4):
                        # V s-tile: out[s128, e512] = sum_d x^T[d,s] wvT[d,e]
                        st = sc * 4 + st4
                        psv = pss.tile([128, 2, SCH], f32, tag="pss")
                        for dt in range(NDT):
                            mm(
                                psv[:, 0, :],
                                xs[:, dt, st4 * 128:(st4 + 1) * 128],
                                wv_sb[:, dt, :],
                                start=(dt == 0), stop=(dt == NDT - 1))
                        nc.vector.tensor_copy(
                            V[:, st, :, 0:HD],
                            psv[:, 0, :].rearrange("p (h v) -> p h v", h=HPG))

                    def transposes(p, qc):
                        # OTsd[s, 2 heads of pair p, hd] -> OTds[d2, s] for
                        # the 4 s-tiles of chunk qc. PE transpose by
                        # default; PSUM->SBUF copies alternate ACT/DVE
                        for qt in range(4):
                            st = 4 * qc + qt
                            if o["tp_dma"]:
                                nc.sync.dma_start_transpose(
                                    OTds[:, p, st * 128:(st + 1) * 128],
                                    OTsd[:, st, 2 * p:2 * p + 2, :])
                                continue
                            pst = pstp.tile([128, 128], bdt, tag="pst")
                            nc.tensor.transpose(
                                pst, OTsd[:, st, 2 * p:2 * p + 2, :], ident)
                            if qt % 2 == 0:
                                nc.scalar.copy(
                                    OTds[:, p, st * 128:(st + 1) * 128], pst)
                            else:
                                nc.vector.tensor_copy(
                                    OTds[:, p, st * 128:(st + 1) * 128], pst)

                    def outproj(st):
                        # one s-tile of the out projection, emitted between
                        # attention blocks so its PE work hides under the
                        # ACT exp backlog; one pss allocation serves both
                        # e-chunks (slot = ec) to halve rotation churn
                        ps_f = pss.tile([128, 2, SCH], f32, tag="pss")
                        for ec in range(D // SCH):
                            for dt in range(NET):
                                mm(
                                    ps_f[:, ec, :],
                                    OTds[:, dt, st * 128:(st + 1) * 128],
                                    wo_sb[:, dt, ec * SCH:(ec + 1) * SCH],
                                    start=(dt == 0), stop=(dt == NET - 1))
                            fo = fop.tile([128, SCH], bdt, tag="fo")
                            nc.vector.tensor_copy(fo, ps_f[:, ec, :])
                            nc.sync.dma_start(
                                out=out[st * 128:(st + 1) * 128,
                                        ec * SCH:(ec + 1) * SCH],
                                in_=fo)

                    def attn_block(p, qc):
                        nkt = 4 * (qc + 1)       # causal: k-tiles 0..nkt-1
                        nfull = nkt - 4          # fully-valid k-tiles
                        q0 = qc * SCH
                        ps_o = pso.tile([128, 2, 4, 128], f32, tag="pso")
                        ets = [None] * nkt
                        # software-pipelined, skew 2: scores/exp of kt issue
                        # before attn@V of kt-2 so PE stays ahead of ACT
                        for kt in range(nkt + 2):
                            if kt < nkt:
                                qlo = max(0, 128 * (kt - nfull))
                                ps_s = pss.tile(
                                    [128, 2, SCH], f32, tag="pss")
                                for j in range(2):
                                    mm(
                                        ps_s[:, j, qlo:],
                                        QKT[64 * j:64 * j + 64, p, 1,
                                            kt * 128:(kt + 1) * 128],
                                        QKT[64 * j:64 * j + 64, p, 0,
                                            q0 + qlo:q0 + SCH],
                                        start=True, stop=True,
                                        tile_position=(64 * j, 0))
                                et_t = etp.tile([128, 2, SCH], bdt, tag="et")
                                ets[kt] = et_t
                                nc.scalar.activation(
                                    et_t[:, :, qlo:], ps_s[:, :, qlo:],
                                    Exp, scale=0.125)
                                if kt >= nfull:
                                    # exact-diagonal 128-wide window mask
                                    nc.vector.tensor_mul(
                                        et_t[:, :, qlo:qlo + 128],
                                        et_t[:, :, qlo:qlo + 128], cm)
                                if with_pad:
                                    nc.vector.tensor_scalar_mul(
                                        et_t[:, :, qlo:], et_t[:, :, qlo:],
                                        pad_sb[:, kt:kt + 1])
                            if kt >= 2:
                                ka = kt - 2
                                qt0 = max(0, ka - nfull)
                                for qt in range(qt0, 4):
                                    for j in range(2):
                                        # one start per PSUM bank (j): the
                                        # zero-region mark makes the first
                                        # touch of every other qt window
                                        # overwrite, later touches accumulate
                                        mm(
                                            ps_o[:, j, qt, 0:HD + 1],
                                            ets[ka][:, j,
                                                    qt * 128:(qt + 1) * 128],
                                            V[:, ka, 2 * p + j, :],
                                            start=(ka == 0 and qt == 0),
                                            stop=(ka == nfull + qt),
                                            skip_group_check=True)
                                if o["norm_perqt"] and ka >= nfull:
                                    # q-tile ka-nfull just got its stop:
                                    # normalize it now so DVE work spreads
                                    # through the block and ps_o frees early
                                    qt = ka - nfull
                                    st = 4 * qc + qt
                                    rec = tmpp.tile([128, 2, 1], f32,
                                                    tag="rec")
                                    nc.vector.reciprocal(
                                        rec, ps_o[:, :, qt, HD:HD + 1])
                                    for j in range(2):
                                        o_ap = OTsd[:, st, 2 * p + j, :]
                                        nc.vector.tensor_scalar_mul(
                                            o_ap, ps_o[:, j, qt, 0:HD],
                                            rec[:, j, :])
                                        if with_bias:
                                            nc.vector.tensor_add(
                                                o_ap, o_ap,
                                                bv_bc[:, 2 * p + j, :])
                        # out projection for the previous q-chunk, one
                        # s-tile per pair-block (spread so ACT's exp
                        # backlog covers the PE time)
                        if qc > 0:
                            outproj(4 * (qc - 1) + p)
                        if not o["norm_perqt"]:
                            # normalize the whole block: one reciprocal,
                            # then per (qt, j) scalar-broadcast multiplies
                            recb = tmpp.tile([128, 2, 4, 1], f32, tag="recb")
                            nc.vector.reciprocal(
                                recb, ps_o[:, :, :, HD:HD + 1])
                            for qt in range(4):
                                st = 4 * qc + qt
                                for j in range(2):
                                    o_ap = OTsd[:, st, 2 * p + j, :]
                                    nc.vector.tensor_scalar_mul(
                                        o_ap, ps_o[:, j, qt, 0:HD],
                                        recb[:, j, qt, :])
                                    if with_bias:
                                        nc.vector.tensor_add(
                                            o_ap, o_ap,
                                            bv_bc[:, 2 * p + j, :])
                        if o["tp_in_block"]:
                            transposes(p, qc)

                    # ---- fused emission schedule ----
                    # phase sc: V units, then per pair qk unit + that
                    # pair's attention for q-chunk sc (its last 4 k-tiles
                    # come from this phase's projections). outproj for
                    # chunk sc-1 rides inside the blocks; the chunk-sc
                    # transposes run in the end-of-phase exp-drain window.
                    for sc in range(NSC):
                        xs = xss[sc]
                        for i in range(NET):
                            proj_v_unit(sc, xs, i)
                        for p in range(NET):
                            proj_qk_unit(sc, xs, p)
                            attn_block(p, sc)
                        if not o["tp_in_block"]:
                            for p in range(NET):
                                transposes(p, sc)
                    for p in range(NET):
                        outproj(4 * 3 + p)
    nc.compile()
    return nc


def prep_in_maps(inputs):
    import ml_dtypes

    bf16 = ml_dtypes.bfloat16
    x = np.asarray(inputs["x"], dtype=np.float32)
    mask = np.asarray(inputs["attention_mask"])
    Wq = np.asarray(inputs["Wq"], dtype=np.float32)
    Wk = np.asarray(inputs["Wk"], dtype=np.float32)
    Wv = np.asarray(inputs["Wv"], dtype=np.float32)
    Wo = np.asarray(inputs["Wo"], dtype=np.float32)
    bq = np.asarray(inputs["bq"], dtype=np.float32)
    bk = np.asarray(inputs["bk"], dtype=np.float32)
    bv = np.asarray(inputs["bv"], dtype=np.float32)
    with_pad = not bool((mask != 0).all())
    with_bias = bool(bq.any() or bk.any() or bv.any())

    # exact-diagonal causal pattern in ST orientation [k, q]: valid iff
    # q >= k within the 128x128 tile; duplicated on axis 1 for the 2 heads
    kk = np.arange(128)[:, None]
    qq = np.arange(128)[None, :]
    cmask = np.repeat(
        (qq >= kk).astype(np.float32)[:, None, :], 2, axis=1).astype(bf16)

    in_maps = []
    for c in range(NCORES):
        b, g = divmod(c, 2)
        es = slice(g * EP, (g + 1) * EP)
        m = {
            "xT": np.ascontiguousarray(x[b].T).astype(bf16),
            "wqT": np.ascontiguousarray(Wq[es, :].T).astype(bf16),
            "wkT": np.ascontiguousarray(Wk[es, :].T).astype(bf16),
            "wvT": np.ascontiguousarray(Wv[es, :].T).astype(bf16),
            "woT": np.ascontiguousarray(Wo[:, es].T).astype(bf16),
            "cmask": cmask,
        }
        if with_bias:
            m["bq"] = np.ascontiguousarray(bq[es])
            m["bk"] = np.ascontiguousarray(bk[es])
            m["bv"] = np.ascontiguousarray(bv[es])
        if with_pad:
            m["padm"] = np.ascontiguousarray(
                mask[b].astype(np.float32).reshape(NST, 128))
        in_maps.append(m)
    return in_maps, (with_pad, with_bias)


def kernel(**inputs):
    from concourse import bass_utils

    in_maps, (with_pad, with_bias) = prep_in_maps(inputs)
    bo = np.asarray(inputs["bo"], dtype=np.float32)

    key = ("prog", with_pad, with_bias)
    if key not in _cache:
        _cache[key] = _build_program(with_pad, with_bias)
    nc = _cache[key]

    res = bass_utils.run_bass_kernel_spmd(nc, in_maps, core_ids=list(range(NCORES)))

    final = np.empty((B, S, D), dtype=np.float32)
    for b in range(B):
        final[b] = (res.results[2 * b]["out"].astype(np.float32)
                    + res.results[2 * b + 1]["out"].astype(np.float32) + bo)
    return final
